# revision 5
# baseline (speedup 1.0000x reference)
"""Trainium2 Bass kernel for a GRU-decoder step (embedding lookup -> GRU cell
-> general attention -> vocab projection), distributed over 8 NeuronCores.

Sharding:
  - GRU cell: sharded over the hidden dim (each core computes 128 of 1024
    hidden units; reads only its slice of W_ih / W_hh). h_new slices are
    AllGathered (transposed, so the gather concatenates on the contraction
    axis) to give every core the full h_new^T for the later matmuls.
  - Attention: sharded over batch (8 of 64 batches per core; reads only its
    slice of enc_outputs). Per-core q rows are picked out of the full q with
    a per-core one-hot selection matrix (SPMD-safe core-dependent slicing).
    context^T slices are AllGathered.
  - Vocab projection (the memory-bound part): W_out sharded over vocab,
    6283 rows per core, streamed from HBM in 4 MB tiles.

All weights are pre-transposed/partition-blocked on the host so that every
big device DMA is a contiguous [128, N] transfer.
"""

import os
import sys

import numpy as np

try:
    import concourse  # noqa: F401
except ImportError:  # pragma: no cover
    sys.path.insert(0, "/opt/trn_rl_repo")

B, S, E, H, V = 64, 128, 512, 1024, 50257
NC_ = 8
BC = B // NC_          # 8 batches per core
HC = H // NC_          # 128 hidden units per core
VPAD = 50264           # V padded to a multiple of 8
VC = VPAD // NC_       # 6283 vocab rows per core
VTS = 512              # vocab tile size (one PSUM bank of fp32)
VT_FULL = VC // VTS    # 12 full tiles
VT_TAIL = VC - VT_FULL * VTS  # 139
EH = E + H             # 1536
CX = EH // 128         # 12 contraction chunks for W_ih
CH = H // 128          # 8 contraction chunks for W_hh / W_attn
K2 = 2 * H             # 2048
KC = K2 // 128         # 16 contraction chunks for W_out

_CACHE = {}


def _build_program():
    import concourse.mybir as mybir
    import concourse.tile as tile
    from concourse import bacc
    from concourse.masks import make_identity

    F32 = mybir.dt.float32
    AX = mybir.AxisListType
    ACT = mybir.ActivationFunctionType
    ALU = mybir.AluOpType

    nc = bacc.Bacc("TRN2", target_bir_lowering=False, debug=False, num_devices=NC_)

    # ---- per-core DRAM inputs (host-prepped layouts) ----
    d_xTg = nc.dram_tensor("xTg", [128, CX * B], F32, kind="ExternalInput")
    d_hprevT = nc.dram_tensor("hprevT", [128, CH * B], F32, kind="ExternalInput")
    d_WihT = nc.dram_tensor("WihT", [128, CX * 384], F32, kind="ExternalInput")
    d_WhhT = nc.dram_tensor("WhhT", [128, CH * 384], F32, kind="ExternalInput")
    d_bih = nc.dram_tensor("bih", [1, 384], F32, kind="ExternalInput")
    d_bhh = nc.dram_tensor("bhh", [1, 384], F32, kind="ExternalInput")
    d_hprevS = nc.dram_tensor("hprevS", [B, HC], F32, kind="ExternalInput")
    d_Wattn = nc.dram_tensor("Wattn", [128, CH * H], F32, kind="ExternalInput")
    d_enc = nc.dram_tensor("enc", [128, BC * H], F32, kind="ExternalInput")
    d_amask = nc.dram_tensor("amask", [BC, S], F32, kind="ExternalInput")
    d_sel = nc.dram_tensor("sel", [B, BC], F32, kind="ExternalInput")
    d_Wmain = nc.dram_tensor("Wmain", [VT_FULL, 128, KC * VTS], F32, kind="ExternalInput")
    d_Wtail = nc.dram_tensor("Wtail", [128, KC * VT_TAIL], F32, kind="ExternalInput")
    d_bout = nc.dram_tensor("bout", [VT_FULL + 1, VTS], F32, kind="ExternalInput")

    # ---- per-core DRAM outputs ----
    d_hnew = nc.dram_tensor("hnew_s", [B, HC], F32, kind="ExternalOutput")
    d_attn = nc.dram_tensor("attn_s", [BC, S], F32, kind="ExternalOutput")
    d_ctxT = nc.dram_tensor("ctxT_s", [H, BC], F32, kind="ExternalOutput")
    d_logits = nc.dram_tensor("logits_s", [B, VC], F32, kind="ExternalOutput")

    with tile.TileContext(nc) as tc:
        with tc.tile_pool(name="cst", bufs=1) as cst, \
             tc.tile_pool(name="wp", bufs=2) as wp, \
             tc.tile_pool(name="lgp", bufs=2) as lgp, \
             tc.tile_pool(name="pmm", bufs=3, space="PSUM") as pmm, \
             tc.tile_pool(name="ptr", bufs=2, space="PSUM") as ptr, \
             tc.tile_pool(name="dram", bufs=1, space="DRAM") as dr:

            # ---- constants ----
            ones = cst.tile([1, 128], F32)
            nc.vector.memset(ones[:], 1.0)
            ident128 = cst.tile([128, 128], F32)
            make_identity(nc, ident128[:])
            ident64 = cst.tile([64, 64], F32)
            make_identity(nc, ident64[:])
            ident16 = cst.tile([16, 16], F32)
            make_identity(nc, ident16[:])
            ident8 = cst.tile([8, 8], F32)
            make_identity(nc, ident8[:])

            # ---- phase A: GRU cell (H-sharded) ----
            # small loads on the gpsimd (SWDGE) queue so they are not stuck
            # behind the big streams on the sync (HWDGE) queue
            xTg_sb = cst.tile([128, CX * B], F32)
            nc.gpsimd.dma_start(out=xTg_sb[:], in_=d_xTg.ap())
            hprevT_sb = cst.tile([128, CH * B], F32)
            nc.gpsimd.dma_start(out=hprevT_sb[:], in_=d_hprevT.ap())
            WihT_sb = cst.tile([128, CX * 384], F32)
            nc.gpsimd.dma_start(out=WihT_sb[:], in_=d_WihT.ap())
            bih_sb = cst.tile([1, 384], F32)
            nc.gpsimd.dma_start(out=bih_sb[:], in_=d_bih.ap())
            bhh_sb = cst.tile([1, 384], F32)
            nc.gpsimd.dma_start(out=bhh_sb[:], in_=d_bhh.ap())
            hprevS_sb = cst.tile([B, HC], F32)
            nc.gpsimd.dma_start(out=hprevS_sb[:], in_=d_hprevS.ap())
            amask_sb = cst.tile([BC, S], F32)
            nc.gpsimd.dma_start(out=amask_sb[:], in_=d_amask.ap())
            sel_sb = cst.tile([B, BC], F32)
            nc.gpsimd.dma_start(out=sel_sb[:], in_=d_sel.ap())
            bout_sb = cst.tile([VT_FULL + 1, VTS], F32)
            nc.gpsimd.dma_start(out=bout_sb[:], in_=d_bout.ap())

            # big loads on the sync queue, in consumption order; W_hh goes
            # through the weight pool (its slot is reused by the W_out stream)
            WhhT_sb = wp.tile([128, CH * 384], F32, tag="w")
            nc.sync.dma_start(out=WhhT_sb[:], in_=d_WhhT.ap())
            Wattn_sb = cst.tile([128, CH * H], F32)
            nc.sync.dma_start(out=Wattn_sb[:], in_=d_Wattn.ap())
            enc_sb = cst.tile([128, BC * H], F32)
            nc.sync.dma_start(out=enc_sb[:], in_=d_enc.ap())

            # gx = x @ W_ih_slice.T + b_ih_slice   [64, 384]
            gx_ps = pmm.tile([64, 384], F32, tag="mm")
            nc.tensor.matmul(out=gx_ps[:], lhsT=ones[:1, :64], rhs=bih_sb[:1, :],
                             start=True, stop=False)
            for c in range(CX):
                nc.tensor.matmul(out=gx_ps[:],
                                 lhsT=xTg_sb[:, c * 64:(c + 1) * 64],
                                 rhs=WihT_sb[:, c * 384:(c + 1) * 384],
                                 start=False, stop=(c == CX - 1))
            # gh = h_prev @ W_hh_slice.T + b_hh_slice   [64, 384]
            gh_ps = pmm.tile([64, 384], F32, tag="mm")
            nc.tensor.matmul(out=gh_ps[:], lhsT=ones[:1, :64], rhs=bhh_sb[:1, :],
                             start=True, stop=False)
            for c in range(CH):
                nc.tensor.matmul(out=gh_ps[:],
                                 lhsT=hprevT_sb[:, c * 64:(c + 1) * 64],
                                 rhs=WhhT_sb[:, c * 384:(c + 1) * 384],
                                 start=False, stop=(c == CH - 1))

            gh_sb = cst.tile([64, 384], F32)
            nc.scalar.copy(gh_sb[:], gh_ps[:])
            # r|z = sigmoid(gx + gh) ; n = tanh(gx_n + r * gh_n)
            rz_sb = cst.tile([64, 256], F32)
            nc.vector.tensor_add(rz_sb[:], gx_ps[:, 0:256], gh_sb[:, 0:256])
            nc.scalar.activation(rz_sb[:], rz_sb[:], ACT.Sigmoid)
            rghn_sb = cst.tile([64, 128], F32)
            nc.vector.tensor_mul(rghn_sb[:], rz_sb[:, 0:128], gh_sb[:, 256:384])
            nc.vector.tensor_add(rghn_sb[:], gx_ps[:, 256:384], rghn_sb[:])
            n_sb = cst.tile([64, 128], F32)
            nc.scalar.activation(n_sb[:], rghn_sb[:], ACT.Tanh)
            # h_new = n + z * (h_prev - n)
            hmn_sb = cst.tile([64, 128], F32)
            nc.vector.tensor_sub(hmn_sb[:], hprevS_sb[:], n_sb[:])
            nc.vector.tensor_mul(hmn_sb[:], rz_sb[:, 128:256], hmn_sb[:])
            hnew_sb = cst.tile([64, 128], F32)
            nc.vector.tensor_add(hnew_sb[:], n_sb[:], hmn_sb[:])
            nc.scalar.dma_start(out=d_hnew.ap(), in_=hnew_sb[:])

            # h_new slice transposed -> AllGather => full h_new^T on each core
            hnewT_ps = ptr.tile([128, 64], F32, tag="tr")
            nc.tensor.transpose(hnewT_ps[:], hnew_sb[:], ident64[:])
            hnewT_sb = cst.tile([128, 64], F32)
            nc.scalar.copy(hnewT_sb[:], hnewT_ps[:])
            ag1_in = dr.tile([128, 64], F32)
            ag1_out = dr.tile([H, 64], F32)
            nc.gpsimd.dma_start(out=ag1_in[:], in_=hnewT_sb[:])
            nc.gpsimd.collective_compute(
                "AllGather", mybir.AluOpType.bypass,
                replica_groups=[list(range(NC_))],
                ins=[ag1_in.opt()], outs=[ag1_out.opt()],
            )
            # xT2 holds [h_new; context]^T as 16 chunks of [128, 64]
            xT2 = cst.tile([128, KC * 64], F32)
            nc.scalar.dma_start(
                out=xT2[:, 0:CH * 64].rearrange("p (c b) -> p c b", b=64),
                in_=ag1_out[:].rearrange("(c p) b -> p c b", p=128),
            )

            # ---- phase B: attention (batch-sharded) ----
            # q = h_new @ W_attn   [64, 1024]  (all 64 batches)
            q_ps = pmm.tile([64, H], F32, tag="mm")
            for hf in range(2):
                for c in range(CH):
                    nc.tensor.matmul(
                        out=q_ps[:, hf * 512:(hf + 1) * 512],
                        lhsT=xT2[:, c * 64:(c + 1) * 64],
                        rhs=Wattn_sb[:, c * H + hf * 512: c * H + hf * 512 + 512],
                        start=(c == 0), stop=(c == CH - 1))
            q_sb = cst.tile([64, H], F32)
            nc.scalar.copy(q_sb[:], q_ps[:])

            # scores[b, s] = sum_h enc[b, s, h] * q[b_global, h]
            # q row for the core's local batch b is extracted with the one-hot
            # selection matrix, broadcast to 128 partitions, then fused
            # multiply+reduce against enc on the vector engine.
            scores0_sb = cst.tile([128, BC], F32)
            rowb_sb = cst.tile([1, H], F32)
            prod_sb = cst.tile([128, H], F32)
            for b in range(BC):
                qb_ps = pmm.tile([128, H], F32, tag="mm")
                for hf in range(2):
                    rh_ps = ptr.tile([1, 512], F32, tag="tr")
                    nc.tensor.matmul(out=rh_ps[:],
                                     lhsT=sel_sb[:, b:b + 1],
                                     rhs=q_sb[:, hf * 512:(hf + 1) * 512],
                                     start=True, stop=True)
                    nc.scalar.copy(rowb_sb[:, hf * 512:(hf + 1) * 512], rh_ps[:])
                for hf in range(2):
                    nc.tensor.matmul(out=qb_ps[:, hf * 512:(hf + 1) * 512],
                                     lhsT=ones[:1, :],
                                     rhs=rowb_sb[:1, hf * 512:(hf + 1) * 512],
                                     start=True, stop=True)
                nc.vector.tensor_mul(prod_sb[:], qb_ps[:],
                                     enc_sb[:, b * H:(b + 1) * H])
                nc.vector.reduce_sum(scores0_sb[:, b:b + 1], prod_sb[:], axis=AX.X)

            # transpose scores to [8, 128], mask, softmax over s
            sc_ps = ptr.tile([BC, S], F32, tag="tr")
            nc.tensor.transpose(sc_ps[:], scores0_sb[:], ident128[:])
            sc_sb = cst.tile([BC, S], F32)
            nc.vector.tensor_add(sc_sb[:], sc_ps[:], amask_sb[:])
            negmx = cst.tile([BC, 1], F32)
            nc.vector.reduce_max(negmx[:], sc_sb[:], axis=AX.X, negate=True)
            e_sb = cst.tile([BC, S], F32)
            nc.scalar.activation(e_sb[:], sc_sb[:], ACT.Exp, bias=negmx[:])
            ssum = cst.tile([BC, 1], F32)
            nc.vector.reduce_sum(ssum[:], e_sb[:], axis=AX.X)
            rsum = cst.tile([BC, 1], F32)
            nc.vector.reciprocal(rsum[:], ssum[:])
            attn_sb = cst.tile([BC, S], F32)
            nc.vector.tensor_scalar_mul(attn_sb[:], e_sb[:], rsum[:])
            nc.scalar.dma_start(out=d_attn.ap(), in_=attn_sb[:])

            # context^T[c*128+p, b] = sum_s enc[b, s, c*128+p] * attn[b, s]
            attnT_ps = ptr.tile([S, BC], F32, tag="tr")
            nc.tensor.transpose(attnT_ps[:], attn_sb[:], ident8[:])
            attnT_sb = cst.tile([S, BC], F32)
            nc.scalar.copy(attnT_sb[:], attnT_ps[:])
            ctxTloc_sb = cst.tile([128, CH * BC], F32)
            for c in range(CH):
                ctxT_ps = ptr.tile([128, BC], F32, tag="tr")
                for b in range(BC):
                    nc.tensor.matmul(
                        out=ctxT_ps[:, b:b + 1],
                        lhsT=enc_sb[:, b * H + c * 128: b * H + (c + 1) * 128],
                        rhs=attnT_sb[:, b:b + 1],
                        start=True, stop=True)
                nc.scalar.copy(ctxTloc_sb[:, c * BC:(c + 1) * BC], ctxT_ps[:])
            nc.scalar.dma_start(
                out=d_ctxT.ap().rearrange("(c p) b -> p c b", p=128),
                in_=ctxTloc_sb[:].rearrange("p (c b) -> p c b", b=BC))

            # AllGather context^T -> [8192, 8], scatter into xT2 chunks 8..15
            ag2_in = dr.tile([H, BC], F32)
            ag2_out = dr.tile([NC_ * H, BC], F32)
            nc.gpsimd.dma_start(
                out=ag2_in[:].rearrange("(c p) b -> p c b", p=128),
                in_=ctxTloc_sb[:].rearrange("p (c b) -> p c b", b=BC))
            nc.gpsimd.collective_compute(
                "AllGather", mybir.AluOpType.bypass,
                replica_groups=[list(range(NC_))],
                ins=[ag2_in.opt()], outs=[ag2_out.opt()],
            )
            ag2_view = ag2_out[:].rearrange("(r c p) b -> c p r b", p=128, c=CH)
            for c in range(CH):
                nc.scalar.dma_start(
                    out=xT2[:, (CH + c) * 64:(CH + c + 1) * 64].rearrange(
                        "p (r b) -> p r b", r=NC_),
                    in_=ag2_view[c])

            # ---- phase C: logits = [h_new, context] @ W_out_slice.T + b_out ----
            for t in range(VT_FULL + 1):
                vt = VTS if t < VT_FULL else VT_TAIL
                w_sb = wp.tile([128, KC * vt], F32, tag="w")
                src = d_Wmain.ap()[t] if t < VT_FULL else d_Wtail.ap()
                nc.sync.dma_start(out=w_sb[:], in_=src)
                # bias row t of bout (one-hot extraction keeps base partition 0)
                br_ps = ptr.tile([1, VTS], F32, tag="tr")
                nc.tensor.matmul(out=br_ps[:, :vt],
                                 lhsT=ident16[0:VT_FULL + 1, t:t + 1],
                                 rhs=bout_sb[:, :vt], start=True, stop=True)
                br_sb = lgp.tile([1, VTS], F32, tag="brow")
                nc.scalar.copy(br_sb[:, :vt], br_ps[:, :vt])
                lg_ps = pmm.tile([64, vt], F32, tag="mm")
                nc.tensor.matmul(out=lg_ps[:], lhsT=ones[:1, :64],
                                 rhs=br_sb[:1, :vt], start=True, stop=False)
                for c in range(KC):
                    nc.tensor.matmul(out=lg_ps[:],
                                     lhsT=xT2[:, c * 64:(c + 1) * 64],
                                     rhs=w_sb[:, c * vt:(c + 1) * vt],
                                     start=False, stop=(c == KC - 1))
                lg_sb = lgp.tile([64, VTS], F32, tag="lg")
                nc.vector.tensor_copy(lg_sb[:, :vt], lg_ps[:])
                nc.scalar.dma_start(out=d_logits.ap()[:, t * VTS: t * VTS + vt],
                                    in_=lg_sb[:, :vt])

    nc.compile()
    return nc


def _prep_inputs(inputs):
    inp = {k: np.asarray(v) for k, v in inputs.items()}
    ids = inp["input_ids"].astype(np.int64)
    emb_table = inp["emb_table"].astype(np.float32, copy=False)
    prev_context = inp["prev_context"].astype(np.float32, copy=False)
    h_prev = np.asarray(inp["hidden"])[0].astype(np.float32, copy=False)
    enc = inp["enc_outputs"].astype(np.float32, copy=False)
    mask = np.asarray(inp["src_mask"])
    W_attn = inp["W_attn"].astype(np.float32, copy=False)
    W_ih = inp["W_ih"].astype(np.float32, copy=False)
    W_hh = inp["W_hh"].astype(np.float32, copy=False)
    b_ih = inp["b_ih"].astype(np.float32, copy=False)
    b_hh = inp["b_hh"].astype(np.float32, copy=False)
    W_out = inp["W_out"].astype(np.float32, copy=False)
    b_out = inp["b_out"].astype(np.float32, copy=False)

    x_g = np.concatenate([emb_table[ids], prev_context], axis=1)  # [64, 1536]
    xTg = np.ascontiguousarray(
        x_g.T.reshape(CX, 128, B).transpose(1, 0, 2)).reshape(128, CX * B)
    hprevT = np.ascontiguousarray(
        h_prev.T.reshape(CH, 128, B).transpose(1, 0, 2)).reshape(128, CH * B)
    Wattn_p = np.ascontiguousarray(
        W_attn.reshape(CH, 128, H).transpose(1, 0, 2)).reshape(128, CH * H)

    W_pad = np.zeros((VPAD, K2), dtype=np.float32)
    W_pad[:V] = W_out
    b_pad = np.zeros((VPAD,), dtype=np.float32)
    b_pad[:V] = b_out

    in_maps = []
    for m in range(NC_):
        rows_ih = np.concatenate(
            [W_ih[g * H + m * HC:(g * H + (m + 1) * HC)] for g in range(3)])  # [384,1536]
        WihT = np.ascontiguousarray(
            rows_ih.T.reshape(CX, 128, 384).transpose(1, 0, 2)).reshape(128, CX * 384)
        rows_hh = np.concatenate(
            [W_hh[g * H + m * HC:(g * H + (m + 1) * HC)] for g in range(3)])  # [384,1024]
        WhhT = np.ascontiguousarray(
            rows_hh.T.reshape(CH, 128, 384).transpose(1, 0, 2)).reshape(128, CH * 384)
        bih_s = np.concatenate(
            [b_ih[g * H + m * HC:(g * H + (m + 1) * HC)] for g in range(3)])[None]
        bhh_s = np.concatenate(
            [b_hh[g * H + m * HC:(g * H + (m + 1) * HC)] for g in range(3)])[None]
        hprevS = np.ascontiguousarray(h_prev[:, m * HC:(m + 1) * HC])
        enc_s = enc[m * BC:(m + 1) * BC]  # [8, 128, 1024]
        enc_p = np.ascontiguousarray(enc_s.transpose(1, 0, 2)).reshape(128, BC * H)
        mask_s = mask[m * BC:(m + 1) * BC].astype(np.float32)
        amask = (mask_s - 1.0) * 1e9  # 0 where visible, -1e9 where masked
        sel = np.zeros((B, BC), dtype=np.float32)
        for b in range(BC):
            sel[m * BC + b, b] = 1.0

        Wc = W_pad[m * VC:(m + 1) * VC]  # [6283, 2048]
        Wmain = np.ascontiguousarray(
            Wc[:VT_FULL * VTS].reshape(VT_FULL, VTS, KC, 128).transpose(0, 3, 2, 1)
        ).reshape(VT_FULL, 128, KC * VTS)
        Wtail = np.ascontiguousarray(
            Wc[VT_FULL * VTS:].reshape(VT_TAIL, KC, 128).transpose(2, 1, 0)
        ).reshape(128, KC * VT_TAIL)
        bout_p = np.zeros((VT_FULL + 1, VTS), dtype=np.float32)
        bout_p.reshape(-1)[:VC] = b_pad[m * VC:(m + 1) * VC]

        in_maps.append({
            "xTg": xTg, "hprevT": hprevT, "WihT": WihT, "WhhT": WhhT,
            "bih": np.ascontiguousarray(bih_s), "bhh": np.ascontiguousarray(bhh_s),
            "hprevS": hprevS, "Wattn": Wattn_p, "enc": enc_p, "amask": amask,
            "sel": sel, "Wmain": Wmain, "Wtail": Wtail, "bout": bout_p,
        })
    return in_maps


def _assemble(results):
    logits = np.concatenate([results[c]["logits_s"] for c in range(NC_)], axis=1)
    logits = np.ascontiguousarray(logits[:, :V])
    h_new = np.concatenate([results[c]["hnew_s"] for c in range(NC_)], axis=1)[None]
    context = np.concatenate([results[c]["ctxT_s"].T for c in range(NC_)], axis=0)
    attn = np.concatenate([results[c]["attn_s"] for c in range(NC_)], axis=0)
    return logits, h_new, context, attn


def _get_program():
    if "nc" not in _CACHE:
        _CACHE["nc"] = _build_program()
    return _CACHE["nc"]


def _run(inputs, trace=False):
    from concourse import bass_utils
    nc = _get_program()
    in_maps = _prep_inputs(inputs)
    res = bass_utils.run_bass_kernel_spmd(
        nc, in_maps, core_ids=list(range(NC_)), trace=trace)
    return _assemble(res.results), res


def kernel(**inputs):
    out, _ = _run(inputs, trace=False)
    return out
